# revision 12
# baseline (speedup 1.0000x reference)
"""Trainium2 Bass kernel for nn_Attention_17222818857675.

Full-input contract: kernel(**inputs) takes the complete tensors, shards
across 8 NeuronCores internally (batch x head-group), runs one SPMD NEFF,
and gathers the full [4, 2048, 1152] output.

Per-core work (b = core//2, g = core%2, heads g*8..g*8+8):
  phase 1: QKV projection in natural [token, dim] layout (bf16 matmuls,
           fp32 PSUM), fused RoPE + per-head RMSNorm (stats computed
           pre-RoPE -- rotation is norm-preserving), PE-transpose of k-hat
           into [dim, token] layout for the scores matmul.
  phase 2: per (q-chunk, head): scores S^T = khT.T @ qhT (bf16), exp on
           ScalarE (fp32 PSUM -> bf16 SBUF), P^T @ V via PE with a ones
           column appended to V giving the softmax denominator for free,
           normalize via partition-broadcast DMA + DVE multiply, then the
           output projection in fp32r. Host sums the two half-head partial
           projections per batch and adds b_proj.
"""

import os
import sys
import types
import numpy as np
import ml_dtypes

# ---------------------------------------------------------------- constants
B, N, C = 4, 2048, 1152
H, DH, HALF = 16, 72, 36
HPC = 8              # heads per core
CPC = HPC * DH       # 576 contraction dims per core
EPS = 1e-6
THETA = 10000.0
NT = N // 128        # 16 token tiles
NCCH = C // 128      # 9 contraction chunks for qkv
QKVC = 432           # qkv output chunk (4 chunks over 1728)
NQKV = (3 * CPC) // QKVC
NJ = 4               # q-chunks of 512
TQ = 512
ECH = 384            # proj output chunk (3 chunks over 1152)
PCB = 5              # proj contraction blocks of 128 (576 -> 4.5 -> 5)

_BF16 = ml_dtypes.bfloat16

# exp2 bit-trick (Schraudolph) constants for DVE-offloaded softmax heads:
# p = bitcast_f32(int32(A_SCALE*s + B_HI + B_LO)) ~= exp(s), rms rel err ~1.8%.
# A_SCALE is folded into the offloaded heads' q-side RMS alpha; B lands in the
# scores via two augmented contraction rows (k rows 72,73 = 1.0).
A_SCALE = float(2.0**23 / np.log(2.0))
B_HI = float(127.0 * 2.0**23)            # bf16-exact
EXP2_C = 0.058                           # sawtooth centering, calibrated on HW
B_LO = float(-EXP2_C * 2.0**23)
OFF_HEADS = (1, 3, 5, 7)                 # heads whose exp runs on DVE+GpSimd
KAUG = DH + 2                            # contraction depth incl. the B rows


# ------------------------------------------------------------------- shims
def _install_shims():
    """axon_hooks module (missing in image) + Tile tail-drain walrus fix."""
    try:
        import antenv.axon_hooks  # noqa: F401
    except ImportError:
        import antenv

        m = types.ModuleType("antenv.axon_hooks")
        m._hook = None
        m.set_axon_ntff_profile_hook = lambda h: setattr(m, "_hook", h)
        m.get_axon_ntff_profile_hook = lambda: m._hook
        sys.modules["antenv.axon_hooks"] = m
        antenv.axon_hooks = m
        try:
            from trn_agent_boot.trn_boot import _ntff_profile_via_ctypes

            so = "/opt/axon/libaxon_pjrt.so"
            if os.path.exists(so):
                hook = _ntff_profile_via_ctypes(so)
                if hook:
                    m.set_axon_ntff_profile_hook(hook)
        except Exception:
            pass

    import concourse.tile as tile

    if os.environ.get("BASSK_LDWOPT") == "1":
        import concourse.bass_utils as bu
        if not getattr(bu, "_ldwopt_patched", False):
            import stat, tempfile
            real = bu.get_walrus_driver()
            shim = os.path.join(tempfile.gettempdir(), "walrus_ldwopt.sh")
            with open(shim, "w") as f:
                f.write('#!/bin/bash\nargs=()\nfor a in "$@"; do\n'
                        '  [[ "$a" == "--enable-ldw-opt=false" ]] && a="--enable-ldw-opt=true"\n'
                        '  args+=("$a")\ndone\nexec "%s" "${args[@]}"\n' % real)
            os.chmod(shim, 0o755)
            bu.get_walrus_driver = lambda: shim
            bu._ldwopt_patched = True

    if getattr(tile.TileContext, "_drain_patched", False):
        return

    def _patched(self, tick_clock, wait_clock):
        nc = self.nc
        gc = tick_clock.global_clock
        for proc, sem in self.sems.allocated().items():
            v = gc[proc]
            if v > 0:
                mult = 16 if sem.name.startswith("DMA") else 1
                nc.sync.wait_ge(sem, v * mult)
        nc.sync.drain()
        nc.all_engine_barrier()
        popped = nc._tile_sem_poison_stack.pop()
        assert popped is self._sem_poison
        nc.clear_and_free_semaphores(list(self.sems.allocated().values()))
        nc.all_engine_barrier()

    tile.TileContext._drain_and_barrier = _patched
    tile.TileContext._drain_patched = True


# ------------------------------------------------------------------ builder
_NC = None


def _build():
    global _NC
    if _NC is not None:
        return _NC
    _install_shims()
    import concourse.bass as bass
    import concourse.mybir as mybir
    import concourse.tile as tile
    from concourse import bacc
    from concourse.masks import make_identity

    f32 = mybir.dt.float32
    f32r = mybir.dt.float32r
    bf16 = mybir.dt.bfloat16
    i32 = mybir.dt.int32
    AF = mybir.ActivationFunctionType
    ALU = mybir.AluOpType

    nc = bacc.Bacc(trn_type="TRN2")

    xT_d = nc.dram_tensor("xT", (128, NT, NCCH, 128), bf16, kind="ExternalInput")
    wqkv_d = nc.dram_tensor("wqkv", (128, NCCH, 3 * CPC), bf16, kind="ExternalInput")
    wproj_d = nc.dram_tensor("wproj", (128, PCB, C), bf16, kind="ExternalInput")
    cosq_d = nc.dram_tensor("cosq", (128, NT, DH), f32, kind="ExternalInput")
    sinq_d = nc.dram_tensor("sinq", (128, NT, DH), f32, kind="ExternalInput")
    cosk_d = nc.dram_tensor("cosk", (128, NT, DH), f32, kind="ExternalInput")
    sink_d = nc.dram_tensor("sink", (128, NT, DH), f32, kind="ExternalInput")
    augk_d = nc.dram_tensor("augk", (2, HPC, N), bf16, kind="ExternalInput")
    augq_d = nc.dram_tensor("augq", (2, HPC, N), bf16, kind="ExternalInput")
    y_d = nc.dram_tensor("y", (N, C), f32, kind="ExternalOutput")

    def APX(base, dims, extra_off=0):
        return bass.AP(tensor=base.tensor, offset=base.offset + extra_off, ap=dims)

    with tile.TileContext(nc) as tc:
        from contextlib import ExitStack

        with ExitStack() as ctx:
            persist = ctx.enter_context(tc.tile_pool(name="persist", bufs=1))
            khT = persist.tile([KAUG, HPC, N], bf16)         # k-hat transposed
            qT = persist.tile([KAUG, HPC, N], bf16)          # q-hat transposed
            vaug = persist.tile([128, NT, HPC, 73], bf16)  # 72 v | ones at 72
            wqkv = persist.tile([128, NCCH, 3 * CPC], bf16)
            wproj = persist.tile([128, PCB, C], bf16)
            cosq = persist.tile([128, NT, DH], f32)
            sinq = persist.tile([128, NT, DH], f32)
            cosk = persist.tile([128, NT, DH], f32)
            sink = persist.tile([128, NT, DH], f32)
            ident = persist.tile([128, 128], bf16)
            eps_q = persist.tile([128, 1], f32)
            eps_k = persist.tile([128, 1], f32)

            make_identity(nc, ident[:])
            nc.vector.memset(eps_q[:], DH * EPS)
            nc.vector.memset(eps_k[:], EPS)
            nc.vector.memset(vaug[:, :, :, 72:73], 1.0)
            # augmented score rows: s' = A*s + (B_HI + B_LO) for bit-trick
            # heads (via DMA -- engine ops can't start at partition 72)
            nc.sync.dma_start(khT[DH : DH + 2, :, :], augk_d[:, :, :])
            nc.sync.dma_start(qT[DH : DH + 2, :, :], augq_d[:, :, :])



            # ------------------------------------------------ phase 1
            with tc.tile_pool(name="p1", bufs=5) as p1, \
                 tc.tile_pool(name="p1s", bufs=2) as p1s, \
                 tc.tile_pool(name="qkps", bufs=1, space="PSUM") as qkps, \
                 tc.tile_pool(name="trps", bufs=2, space="PSUM") as trps:
                pend = []
                xts = {}

                def load_xt(dst, _p):
                    # split tile loads across DMA engines (295KB on one
                    # engine takes 13us; halves land in 6.5)
                    for _c3 in range(0, NCCH, 3):
                        nc.sync.dma_start(dst[:, _c3 : _c3 + 3],
                                          xT_d[:, _p, _c3 : _c3 + 3])

                for _p in range(4):
                    xts[_p] = p1.tile([128, NCCH, 128], bf16, tag="xt",
                                      name="xt_pre%d" % _p)
                    load_xt(xts[_p], _p)
                    # rope-table chunks ride the sync queue between the xt
                    # prefetches (tile it needs table chunk it//4)
                    _q = _p * 4
                    nc.sync.dma_start(cosq[:, _q : _q + 4], cosq_d[:, _q : _q + 4])
                    nc.sync.dma_start(sinq[:, _q : _q + 4], sinq_d[:, _q : _q + 4])
                    nc.sync.dma_start(cosk[:, _q : _q + 4], cosk_d[:, _q : _q + 4])
                    nc.sync.dma_start(sink[:, _q : _q + 4], sink_d[:, _q : _q + 4])
                for _c in range(NCCH):
                    # halve the 442KB weight chunks; spread the ~0.9us-each
                    # issue cost across the scalar and gpsimd queues
                    nc.scalar.dma_start(
                        wqkv[:, _c, 0:864], wqkv_d[:, _c, 0:864])
                    nc.gpsimd.dma_start(
                        wqkv[:, _c, 864:1728], wqkv_d[:, _c, 864:1728])
                for it in range(NT):
                    if it in xts:
                        xt = xts.pop(it)
                    else:
                        xt = p1.tile([128, NCCH, 128], bf16, tag="xt")
                        load_xt(xt, it)

                    qk = p1.tile([128, 2 * CPC], f32, tag="qk")
                    pss = [qkps.tile([128, QKVC], f32, tag="qkvps%d" % _n,
                                     name="qkvps%d_%d" % (_n, it))
                           for _n in range(NQKV)]
                    for cch in range(NCCH):
                        for nch in range(NQKV):
                            nc.tensor.matmul(
                                pss[nch][:],
                                lhsT=xt[:, cch, :],
                                rhs=wqkv[:, cch, nch * QKVC : (nch + 1) * QKVC],
                                start=(cch == 0),
                                stop=(cch == NCCH - 1),
                            )
                    for nch in range(NQKV):
                        ps = pss[nch]
                        if nch < 2:      # pure q / q+k cols
                            nc.scalar.copy(qk[:, nch * QKVC : (nch + 1) * QKVC], ps[:])
                        elif nch == 2:   # 288 k cols + heads 0-1 of v
                            nc.scalar.copy(qk[:, 864:1152], ps[:, 0:288])
                            nc.scalar.copy(
                                vaug[:, it, 0:2, 0:DH],
                                ps[:, 288:432].rearrange("p (h d) -> p h d", h=2),
                            )
                        else:            # heads 2-7 of v
                            nc.scalar.copy(
                                vaug[:, it, 2:8, 0:DH],
                                ps[:].rearrange("p (h d) -> p h d", h=6),
                            )

                    # RMS stats (pre-RoPE; rotation preserves norms)
                    sq = p1s.tile([128, 2 * CPC], f32, tag="sq")
                    nc.scalar.activation(sq[:], qk[:], AF.Square)
                    ms = p1s.tile([128, 16], f32, tag="ms")
                    nc.vector.tensor_reduce(
                        ms[:], sq[:].rearrange("p (g d) -> p g d", g=16),
                        axis=mybir.AxisListType.X, op=ALU.add,
                    )
                    rms = p1s.tile([128, 16], f32, tag="rms")
                    # q: 1/sqrt(sum + DH*eps) also folds the DH**-0.5 score scale
                    nc.scalar.activation(rms[:, 0:8], ms[:, 0:8], AF.Sqrt, bias=eps_q[:])
                    # k: 1/sqrt(sum/DH + eps)
                    nc.scalar.activation(rms[:, 8:16], ms[:, 8:16], AF.Sqrt,
                                         bias=eps_k[:], scale=1.0 / DH)
                    alpha = p1s.tile([128, 16], f32, tag="alpha")
                    nc.vector.reciprocal(alpha[:], rms[:])
                    # fold the exp2 bit-trick scale into offloaded heads' q-alpha
                    assert OFF_HEADS == (1, 3, 5, 7)
                    off_ap = APX(alpha[:], [alpha[:].ap[0], [2, 4]], 1)
                    nc.vector.tensor_scalar_mul(off_ap, off_ap, A_SCALE)

                    # RoPE + alpha scaling.  qk cols: q = [0:576), k = [576:1152)
                    def rope(base_off, cos_t, sin_t, alpha_sl, out_sl, eng):
                        tmp = p1s.tile([128, CPC], f32, tag="ropetmp%d" % base_off)
                        rot = p1s.tile([128, CPC], f32, tag="roterot%d" % base_off)
                        qk0 = qk[:, base_off : base_off + CPC]
                        p_tmp, p_qk = tmp[:].ap[0], qk0.ap[0]
                        p_cos, p_sin = cos_t.ap[0], sin_t.ap[0]
                        p_al, p_out = alpha_sl.ap[0], out_sl.ap[0]
                        # tmp[h,0:36] = x2 * (-sin) ; tmp[h,36:72] = x1 * (+sin)
                        eng.tensor_tensor(
                            APX(tmp[:], [p_tmp, [DH, HPC], [1, HALF]]),
                            APX(qk0, [p_qk, [DH, HPC], [1, HALF]], HALF),
                            APX(sin_t, [p_sin, [0, HPC], [1, HALF]]),
                            op=ALU.mult,
                        )
                        eng.tensor_tensor(
                            APX(tmp[:], [p_tmp, [DH, HPC], [1, HALF]], HALF),
                            APX(qk0, [p_qk, [DH, HPC], [1, HALF]]),
                            APX(sin_t, [p_sin, [0, HPC], [1, HALF]], HALF),
                            op=ALU.mult,
                        )
                        eng.tensor_tensor(
                            rot[:].rearrange("p (h d) -> p h d", h=HPC),
                            qk0.rearrange("p (h d) -> p h d", h=HPC),
                            APX(cos_t, [p_cos, [0, HPC], [1, DH]]),
                            op=ALU.mult,
                        )
                        eng.tensor_tensor(rot[:], rot[:], tmp[:], op=ALU.add)
                        eng.tensor_tensor(
                            out_sl.rearrange("p (h d) -> p h d", h=HPC),
                            rot[:].rearrange("p (h d) -> p h d", h=HPC),
                            APX(alpha_sl, [p_al, [1, HPC], [0, DH]]),
                            op=ALU.mult,
                        )

                    qhat_t = p1s.tile([128, CPC], bf16, tag="qhat")
                    rope(0, cosq[:, it, :], sinq[:, it, :], alpha[:, 0:8],
                         qhat_t[:], nc.vector)
                    khat = p1s.tile([128, CPC], bf16, tag="khat")
                    rope(CPC, cosk[:, it, :], sink[:, it, :], alpha[:, 8:16],
                         khat[:], nc.gpsimd)
                    if len(pend) > 1:
                        emit_ktr(*pend.pop(0))

                    # PE-transpose k-hat per head -> khT bf16 (deferred one
                    # tile so PE never waits on this tile's rope chain)
                    def emit_ktr(it_, khat_, qhat_ref_):
                        for dst, nat, cptag in ((khT, khat_, 0), (qT, qhat_ref_, 1)):
                            for hb in (0, 4):
                                tp = trps.tile([DH, 4, 128], bf16,
                                               tag="ktr%d" % cptag,
                                               name="ktr%d_%d_%d" % (cptag, it_, hb))
                                for h4 in range(4):
                                    src_ap = nat[:, (hb + h4) * DH : (hb + h4 + 1) * DH]
                                    nc.tensor.transpose(tp[:, h4, :], src_ap, ident[:])
                                if cptag == 0:
                                    nc.scalar.copy(
                                        dst[0:DH, hb : hb + 4, it_ * 128 : (it_ + 1) * 128],
                                        tp[:],
                                    )
                                else:
                                    nc.vector.tensor_copy(
                                        dst[0:DH, hb : hb + 4, it_ * 128 : (it_ + 1) * 128],
                                        tp[:],
                                    )
                    pend.append((it, khat, qhat_t))

                for _args in pend:
                    emit_ktr(*_args)

            for _cb in range(PCB):
                nc.gpsimd.dma_start(wproj[:, _cb], wproj_d[:, _cb])

            # ------------------------------------------------ phase 2
            with tc.tile_pool(name="p2", bufs=2) as p2, \
                 tc.tile_pool(name="p2o", bufs=4) as p2o, \
                 tc.tile_pool(name="pint", bufs=3) as pints, \
                 tc.tile_pool(name="sps", bufs=2, space="PSUM") as sps, \
                 tc.tile_pool(name="pvps", bufs=2, space="PSUM") as pvps, \
                 tc.tile_pool(name="yps", bufs=2, space="PSUM") as yps, \
                 tc.tile_pool(name="dram", bufs=1, space="DRAM") as dpool:
                rec_dram = dpool.tile([NJ, HPC, TQ], f32)
                rec2_dram = dpool.tile([NJ, HPC, TQ], f32)

                def emit_proj_group(proj_in_, j_, ts, e):
                    yp = yps.tile([128, ECH], f32, tag="yp",
                                  name="yp%d_%d_%d" % (j_, ts, e))
                    for cb in range(PCB):
                        rows = 128 if cb < PCB - 1 else CPC - 128 * (PCB - 1)
                        nc.tensor.matmul(
                            yp[:],
                            lhsT=proj_in_[0:rows, cb, ts * 128 : (ts + 1) * 128],
                            rhs=wproj[0:rows, cb, e * ECH : (e + 1) * ECH],
                            start=(cb == 0), stop=(cb == PCB - 1),
                        )
                    ysb = p2o.tile([128, ECH], f32, tag="ysb",
                                   name="ysb%d_%d_%d" % (j_, ts, e))
                    nc.vector.tensor_copy(ysb[:], yp[:])
                    for _yh in range(2):
                        nc.sync.dma_start(
                            y_d[j_ * TQ + ts * 128 : j_ * TQ + (ts + 1) * 128,
                                e * ECH + _yh * (ECH // 2)
                                : e * ECH + (_yh + 1) * (ECH // 2)],
                            ysb[:, _yh * (ECH // 2) : (_yh + 1) * (ECH // 2)],
                        )

                proj_pend = []
                proj_ins = {}
                pv_tiles = {}

                def finish_head(j, h):
                    # normalize: row 96 of pv is the softmax denominator
                    pv = pv_tiles.pop((j, h))
                    proj_in = proj_ins[j]
                    pvs = p2o.tile([73, TQ], f32, tag="pvs")
                    nc.vector.tensor_copy(pvs[:], pv[:])
                    nrm = p2o.tile([73, TQ], f32, tag="nrm")
                    # reshape the denominator row to [8,64] via DRAM so the
                    # microcoded reciprocal runs 8-wide (~6x cheaper on DVE)
                    nc.sync.dma_start(rec_dram[j, h, :], pvs[72:73, :])
                    r8 = p2o.tile([8, 64], f32, tag="r8")
                    r8o = p2o.tile([8, 64], f32, tag="r8o")
                    nc.sync.dma_start(
                        r8[:], APX(rec_dram[j, h, :], [[64, 8], [1, 64]]))
                    nc.vector.reciprocal(r8o[:], r8[:])
                    nc.sync.dma_start(
                        APX(rec2_dram[j, h, :], [[64, 8], [1, 64]]), r8o[:])
                    bc = nrm[0:DH, :]
                    nc.sync.dma_start(
                        bc,
                        APX(rec2_dram[j, h, :], [[0, DH], [1, TQ]]),
                    )
                    outT = p2o.tile([DH, TQ], bf16, tag="outT")
                    nc.gpsimd.tensor_tensor(outT[:], pvs[0:DH, :], bc,
                                            op=ALU.mult)
                    # repack head rows into 128-row proj blocks (SBUF->SBUF DMA)
                    r0 = h * DH
                    cb0, off0 = divmod(r0, 128)
                    n0 = min(DH, 128 - off0)
                    nc.gpsimd.dma_start(
                        proj_in[off0 : off0 + n0, cb0, :], outT[0:n0, :]
                    )
                    if n0 < DH:
                        nc.gpsimd.dma_start(
                            proj_in[0 : DH - n0, cb0 + 1, :], outT[n0:DH, :]
                        )
                    if h == HPC - 1:
                        # queue this q-chunk's projection; drained next chunk
                        for ts in range(4):
                            for e in range(C // ECH):
                                proj_pend.append((proj_in, j, ts, e))

                def emit_pv(j, h, gg, pbuf):
                    pv = pv_tiles[(j, h)]
                    for ii in range(2):
                        i = gg * 2 + ii
                        nc.tensor.matmul(
                            pv[:],
                            lhsT=vaug[:, i, h, :],
                            rhs=pbuf[:, ii, :],
                            start=(i == 0), stop=(i == 15),
                            skip_group_check=True,
                        )
                    if gg == 7:
                        finish_head(j, h)

                # pair-interleaved software-pipelined stream: heads (2hp,
                # 2hp+1) alternate per k-pair so the even head's exp
                # (ScalarE) and the odd head's exp2 bit-trick (DVE int32
                # convert + GpSimd bitcast copy) run on disjoint engines.
                # PV for step n is deferred 3 steps to cover the act-chain
                # latency, keeping the PE streaming back-to-back.
                stream = []
                for j in range(NJ):
                    for hp in range(HPC // 2):
                        for gg in range(8):
                            stream.append((j, 2 * hp, gg))
                            stream.append((j, 2 * hp + 1, gg))
                pend_pv = []
                for (j, h, gg) in stream:
                    if h == 0 and gg == 0:
                        proj_ins[j] = p2.tile([128, PCB, TQ], bf16,
                                              tag="proj_in",
                                              name="proj_in%d" % j)
                    if gg == 0:
                        pv_tiles[(j, h)] = pvps.tile(
                            [73, TQ], f32, tag="pv", name="pv%d_%d" % (j, h))
                    sp = sps.tile([128, 2, TQ], f32, tag="sp")
                    pbuf = p2o.tile([128, 2, TQ], bf16, tag="pbuf")
                    koff = KAUG if h in OFF_HEADS else DH
                    for ii in range(2):
                        i = gg * 2 + ii
                        nc.tensor.matmul(
                            sp[:, ii, :],
                            lhsT=khT[0:koff, h, i * 128 : (i + 1) * 128],
                            rhs=qT[0:koff, h, j * TQ : (j + 1) * TQ],
                            start=True, stop=True,
                        )
                    if h in OFF_HEADS:
                        pi = pints.tile([128, 2, TQ], i32, tag="pi")
                        nc.vector.tensor_copy(pi[:], sp[:])
                        nc.gpsimd.tensor_scalar_mul(
                            pbuf[:].rearrange("p a b -> p (a b)"),
                            pi.bitcast(f32)[:].rearrange("p a b -> p (a b)"),
                            1.0,
                        )
                    else:
                        nc.scalar.activation(
                            pbuf[:].rearrange("p a b -> p (a b)"),
                            sp[:].rearrange("p a b -> p (a b)"),
                            AF.Exp,
                        )
                    pend_pv.append((j, h, gg, pbuf))
                    if len(pend_pv) > 3:
                        emit_pv(*pend_pv.pop(0))
                    # spread deferred proj groups into the stream
                    if h % 2 == 1 and gg in (2, 4, 6) and proj_pend:
                        emit_proj_group(*proj_pend.pop(0))
                for _a in pend_pv:
                    emit_pv(*_a)
                # dummy matmuls bridge the ~8us norm-chain latency before the
                # last chunk's proj groups, keeping the PE at full p-state
                for _w in range(16):
                    warm = sps.tile([128, 2, TQ], f32, tag="sp",
                                    name="warm%d" % _w)
                    nc.tensor.matmul(
                        warm[:, 0, :],
                        lhsT=khT[0:DH, 0, 0:128],
                        rhs=qT[0:DH, 0, 0:TQ],
                        start=True, stop=True,
                    )
                for args in proj_pend:
                    emit_proj_group(*args)

    nc.compile()
    _NC = nc
    return nc


# -------------------------------------------------------------- host prep
def _prep_shards(x, w_qkv, w_proj, q_norm_w, k_norm_w):
    inv_freq = 1.0 / (THETA ** (np.arange(HALF, dtype=np.float32) / HALF))
    ang = np.arange(N, dtype=np.float32)[:, None] * inv_freq[None, :]
    cos_t, sin_t = np.cos(ang), np.sin(ang)  # [N, 36]

    def rope_tabs(w):
        # cos2[t, j] = cos(ang) * w[j] (both halves); sin2s = [-sin, +sin] * w
        c2 = np.concatenate([cos_t * w[:HALF], cos_t * w[HALF:]], axis=1)
        s2 = np.concatenate([-sin_t * w[:HALF], sin_t * w[HALF:]], axis=1)
        tile_form = lambda a: np.ascontiguousarray(
            a.reshape(NT, 128, DH).transpose(1, 0, 2)
        ).astype(np.float32)
        return tile_form(c2), tile_form(s2)

    cq, sq_ = rope_tabs(np.asarray(q_norm_w, np.float32))
    ck, sk = rope_tabs(np.asarray(k_norm_w, np.float32))

    augk = np.ones((2, HPC, N), dtype=_BF16)
    augq = np.empty((2, HPC, N), dtype=_BF16)
    augq[0] = _BF16(B_HI)
    augq[1] = _BF16(B_LO)

    xTs = []
    for b in range(B):
        xt = np.ascontiguousarray(x[b].T)  # [1152, 2048]
        xt = xt.reshape(NCCH, 128, NT, 128).transpose(1, 2, 0, 3)
        xTs.append(np.ascontiguousarray(xt).astype(_BF16))

    in_maps = []
    for core in range(8):
        b, g = divmod(core, 2)
        h0 = g * HPC
        rq = w_qkv[h0 * DH : h0 * DH + CPC]                     # [576, 1152]
        rk = w_qkv[C + h0 * DH : C + h0 * DH + CPC]
        rv = w_qkv[2 * C + h0 * DH : 2 * C + h0 * DH + CPC]
        wk = np.concatenate([rq, rk, rv], axis=0).T             # [1152, 1728]
        wk = wk.reshape(NCCH, 128, 3 * CPC).transpose(1, 0, 2)
        wk = np.ascontiguousarray(wk).astype(_BF16)

        wp = w_proj[:, g * CPC : (g + 1) * CPC].T               # [576, 1152]
        wp = np.concatenate(
            [wp, np.zeros((PCB * 128 - CPC, C), np.float32)], axis=0
        )
        wp = wp.reshape(PCB, 128, C).transpose(1, 0, 2)
        wp = np.ascontiguousarray(wp).astype(_BF16)

        in_maps.append({
            "xT": xTs[b], "wqkv": wk, "wproj": wp,
            "cosq": cq, "sinq": sq_, "cosk": ck, "sink": sk,
            "augk": augk, "augq": augq,
        })
    return in_maps


def kernel(x, w_qkv, w_proj, b_proj, q_norm_w, k_norm_w):
    x = np.asarray(x, np.float32)
    w_qkv = np.asarray(w_qkv, np.float32)
    w_proj = np.asarray(w_proj, np.float32)
    b_proj = np.asarray(b_proj, np.float32)

    nc = _build()
    from concourse.bass_utils import run_bass_kernel_spmd

    in_maps = _prep_shards(x, w_qkv, w_proj, q_norm_w, k_norm_w)
    res = run_bass_kernel_spmd(nc, in_maps, core_ids=list(range(8)))
    y = np.empty((B, N, C), np.float32)
    for b in range(B):
        y[b] = res.results[2 * b]["y"] + res.results[2 * b + 1]["y"] + b_proj
    return y



# revision 17
# speedup vs baseline: 4.2154x; 4.2154x over previous
"""Trainium2 Bass kernel for nn_Attention_17222818857675.

Full-input contract: kernel(**inputs) takes the complete tensors, shards
across 8 NeuronCores internally (batch x head-group), runs one SPMD NEFF,
and gathers the full [4, 2048, 1152] output.

Per-core work (b = core//2, g = core%2, heads g*8..g*8+8):
  phase 1: QKV projection in natural [token, dim] layout (bf16 matmuls,
           fp32 PSUM), fused RoPE + per-head RMSNorm (stats computed
           pre-RoPE -- rotation is norm-preserving), PE-transpose of k-hat
           into [dim, token] layout for the scores matmul.
  phase 2: per (q-chunk, head): scores S^T = khT.T @ qhT (bf16), exp on
           ScalarE (fp32 PSUM -> bf16 SBUF), P^T @ V via PE with a ones
           column appended to V giving the softmax denominator for free,
           normalize via partition-broadcast DMA + DVE multiply, then the
           output projection in fp32r. Host sums the two half-head partial
           projections per batch and adds b_proj.
"""

import os
import sys
import types
import numpy as np
import ml_dtypes

# ---------------------------------------------------------------- constants
B, N, C = 4, 2048, 1152
H, DH, HALF = 16, 72, 36
HPC = 8              # heads per core
CPC = HPC * DH       # 576 contraction dims per core
EPS = 1e-6
THETA = 10000.0
NT = N // 128        # 16 token tiles
NCCH = C // 128      # 9 contraction chunks for qkv
QKVC = 432           # qkv output chunk (4 chunks over 1728)
NQKV = (3 * CPC) // QKVC
NJ = 4               # q-chunks of 512
TQ = 512
ECH = 384            # proj output chunk (3 chunks over 1152)
PCB = 5              # proj contraction blocks of 128 (576 -> 4.5 -> 5)

_BF16 = ml_dtypes.bfloat16

# 16-bit exp2 bit-trick (Schraudolph) for DVE-offloaded softmax heads:
# bitcast_bf16(int16(A_SCALE*s + B_HI + B_LO)) ~= exp(s), rms rel err ~1.8%.
# A_SCALE is folded into the offloaded heads' q-side RMS alpha; B lands in the
# scores via two augmented contraction rows (k rows 72,73 = 1.0), so the DVE
# only does one fp32->int16 converting copy per step (round-to-nearest on HW).
A_SCALE = float(2.0**7 / np.log(2.0))
B_HI = float(127.0 * 2.0**7)             # bf16-exact
EXP2_C = 7.375                           # sawtooth centering (bf16-exact)
B_LO = -EXP2_C
OFF_HEADS = (1, 3, 5, 7)                 # heads whose exp runs on DVE
KAUG = DH + 2                            # contraction depth incl. the B rows


# ------------------------------------------------------------------- shims
def _install_shims():
    """axon_hooks module (missing in image) + Tile tail-drain walrus fix."""
    try:
        import antenv.axon_hooks  # noqa: F401
    except ImportError:
        import antenv

        m = types.ModuleType("antenv.axon_hooks")
        m._hook = None
        m.set_axon_ntff_profile_hook = lambda h: setattr(m, "_hook", h)
        m.get_axon_ntff_profile_hook = lambda: m._hook
        sys.modules["antenv.axon_hooks"] = m
        antenv.axon_hooks = m
        try:
            from trn_agent_boot.trn_boot import _ntff_profile_via_ctypes

            so = "/opt/axon/libaxon_pjrt.so"
            if os.path.exists(so):
                hook = _ntff_profile_via_ctypes(so)
                if hook:
                    m.set_axon_ntff_profile_hook(hook)
        except Exception:
            pass

    import concourse.tile as tile

    if os.environ.get("BASSK_LDWOPT") == "1":
        import concourse.bass_utils as bu
        if not getattr(bu, "_ldwopt_patched", False):
            import stat, tempfile
            real = bu.get_walrus_driver()
            shim = os.path.join(tempfile.gettempdir(), "walrus_ldwopt.sh")
            with open(shim, "w") as f:
                f.write('#!/bin/bash\nargs=()\nfor a in "$@"; do\n'
                        '  [[ "$a" == "--enable-ldw-opt=false" ]] && a="--enable-ldw-opt=true"\n'
                        '  args+=("$a")\ndone\nexec "%s" "${args[@]}"\n' % real)
            os.chmod(shim, 0o755)
            bu.get_walrus_driver = lambda: shim
            bu._ldwopt_patched = True

    if getattr(tile.TileContext, "_drain_patched", False):
        return

    def _patched(self, tick_clock, wait_clock):
        nc = self.nc
        gc = tick_clock.global_clock
        for proc, sem in self.sems.allocated().items():
            v = gc[proc]
            if v > 0:
                mult = 16 if sem.name.startswith("DMA") else 1
                nc.sync.wait_ge(sem, v * mult)
        nc.sync.drain()
        nc.all_engine_barrier()
        popped = nc._tile_sem_poison_stack.pop()
        assert popped is self._sem_poison
        nc.clear_and_free_semaphores(list(self.sems.allocated().values()))
        nc.all_engine_barrier()

    tile.TileContext._drain_and_barrier = _patched
    tile.TileContext._drain_patched = True


# ------------------------------------------------------------------ builder
_NC = None


def _build():
    global _NC
    if _NC is not None:
        return _NC
    _install_shims()
    import concourse.bass as bass
    import concourse.mybir as mybir
    import concourse.tile as tile
    from concourse import bacc
    from concourse.masks import make_identity

    f32 = mybir.dt.float32
    f32r = mybir.dt.float32r
    bf16 = mybir.dt.bfloat16
    i16 = mybir.dt.int16
    AF = mybir.ActivationFunctionType
    ALU = mybir.AluOpType

    nc = bacc.Bacc(trn_type="TRN2")

    xT_d = nc.dram_tensor("xT", (128, NT, NCCH, 128), bf16, kind="ExternalInput")
    wqkv_d = nc.dram_tensor("wqkv", (128, NCCH, 3 * CPC), bf16, kind="ExternalInput")
    wproj_d = nc.dram_tensor("wproj", (128, PCB, C), bf16, kind="ExternalInput")
    cosq_d = nc.dram_tensor("cosq", (128, NT, DH), f32, kind="ExternalInput")
    sinq_d = nc.dram_tensor("sinq", (128, NT, DH), f32, kind="ExternalInput")
    cosk_d = nc.dram_tensor("cosk", (128, NT, DH), f32, kind="ExternalInput")
    sink_d = nc.dram_tensor("sink", (128, NT, DH), f32, kind="ExternalInput")
    augk_d = nc.dram_tensor("augk", (2, HPC, N), bf16, kind="ExternalInput")
    augq_d = nc.dram_tensor("augq", (2, HPC, N), bf16, kind="ExternalInput")
    y_d = nc.dram_tensor("y", (N, C), f32, kind="ExternalOutput")

    def APX(base, dims, extra_off=0):
        return bass.AP(tensor=base.tensor, offset=base.offset + extra_off, ap=dims)

    with tile.TileContext(nc) as tc:
        from contextlib import ExitStack

        with ExitStack() as ctx:
            persist = ctx.enter_context(tc.tile_pool(name="persist", bufs=1))
            khT = persist.tile([KAUG, HPC, N], bf16)         # k-hat transposed
            qT = persist.tile([KAUG, HPC, N], bf16)          # q-hat transposed
            vaug = persist.tile([128, NT, HPC, 73], bf16)  # 72 v | ones at 72
            wqkv = persist.tile([128, NCCH, 3 * CPC], bf16)
            wproj = persist.tile([128, PCB, C], bf16)
            cosq = persist.tile([128, NT, DH], f32)
            sinq = persist.tile([128, NT, DH], f32)
            cosk = persist.tile([128, NT, DH], f32)
            sink = persist.tile([128, NT, DH], f32)
            ident = persist.tile([128, 128], bf16)
            eps_q = persist.tile([128, 1], f32)
            eps_k = persist.tile([128, 1], f32)

            make_identity(nc, ident[:])
            nc.vector.memset(eps_q[:], DH * EPS)
            nc.vector.memset(eps_k[:], EPS)
            nc.vector.memset(vaug[:, :, :, 72:73], 1.0)
            # augmented score rows: s' = A*s + (B_HI + B_LO) for bit-trick
            # heads (via DMA -- engine ops can't start at partition 72)
            nc.sync.dma_start(khT[DH : DH + 2, :, :], augk_d[:, :, :])
            nc.sync.dma_start(qT[DH : DH + 2, :, :], augq_d[:, :, :])



            # ------------------------------------------------ phase 1
            with tc.tile_pool(name="p1", bufs=5) as p1, \
                 tc.tile_pool(name="p1s", bufs=2) as p1s, \
                 tc.tile_pool(name="qkps", bufs=1, space="PSUM") as qkps, \
                 tc.tile_pool(name="trps", bufs=2, space="PSUM") as trps:
                pend = []
                xts = {}

                def load_xt(dst, _p):
                    # split tile loads across DMA engines (295KB on one
                    # engine takes 13us; halves land in 6.5)
                    for _c3 in range(0, NCCH, 3):
                        nc.sync.dma_start(dst[:, _c3 : _c3 + 3],
                                          xT_d[:, _p, _c3 : _c3 + 3])

                for _p in range(4):
                    xts[_p] = p1.tile([128, NCCH, 128], bf16, tag="xt",
                                      name="xt_pre%d" % _p)
                    load_xt(xts[_p], _p)
                    # rope-table chunks ride the sync queue between the xt
                    # prefetches (tile it needs table chunk it//4)
                    _q = _p * 4
                    nc.sync.dma_start(cosq[:, _q : _q + 4], cosq_d[:, _q : _q + 4])
                    nc.sync.dma_start(sinq[:, _q : _q + 4], sinq_d[:, _q : _q + 4])
                    nc.sync.dma_start(cosk[:, _q : _q + 4], cosk_d[:, _q : _q + 4])
                    nc.sync.dma_start(sink[:, _q : _q + 4], sink_d[:, _q : _q + 4])
                for _c in range(NCCH):
                    # halve the 442KB weight chunks; spread the ~0.9us-each
                    # issue cost across the scalar and gpsimd queues
                    nc.scalar.dma_start(
                        wqkv[:, _c, 0:864], wqkv_d[:, _c, 0:864])
                    nc.gpsimd.dma_start(
                        wqkv[:, _c, 864:1728], wqkv_d[:, _c, 864:1728])
                for it in range(NT):
                    if it in xts:
                        xt = xts.pop(it)
                    else:
                        xt = p1.tile([128, NCCH, 128], bf16, tag="xt")
                        load_xt(xt, it)

                    qk = p1.tile([128, 2 * CPC], f32, tag="qk")
                    pss = [qkps.tile([128, QKVC], f32, tag="qkvps%d" % _n,
                                     name="qkvps%d_%d" % (_n, it))
                           for _n in range(NQKV)]
                    for cch in range(NCCH):
                        for nch in range(NQKV):
                            nc.tensor.matmul(
                                pss[nch][:],
                                lhsT=xt[:, cch, :],
                                rhs=wqkv[:, cch, nch * QKVC : (nch + 1) * QKVC],
                                start=(cch == 0),
                                stop=(cch == NCCH - 1),
                            )
                    for nch in range(NQKV):
                        ps = pss[nch]
                        if nch < 2:      # pure q / q+k cols
                            nc.scalar.copy(qk[:, nch * QKVC : (nch + 1) * QKVC], ps[:])
                        elif nch == 2:   # 288 k cols + heads 0-1 of v
                            nc.scalar.copy(qk[:, 864:1152], ps[:, 0:288])
                            nc.scalar.copy(
                                vaug[:, it, 0:2, 0:DH],
                                ps[:, 288:432].rearrange("p (h d) -> p h d", h=2),
                            )
                        else:            # heads 2-7 of v
                            nc.scalar.copy(
                                vaug[:, it, 2:8, 0:DH],
                                ps[:].rearrange("p (h d) -> p h d", h=6),
                            )

                    # RMS stats (pre-RoPE; rotation preserves norms)
                    sq = p1s.tile([128, 2 * CPC], f32, tag="sq")
                    nc.scalar.activation(sq[:], qk[:], AF.Square)
                    ms = p1s.tile([128, 16], f32, tag="ms")
                    nc.vector.tensor_reduce(
                        ms[:], sq[:].rearrange("p (g d) -> p g d", g=16),
                        axis=mybir.AxisListType.X, op=ALU.add,
                    )
                    rms = p1s.tile([128, 16], f32, tag="rms")
                    # q: 1/sqrt(sum + DH*eps) also folds the DH**-0.5 score scale
                    nc.scalar.activation(rms[:, 0:8], ms[:, 0:8], AF.Sqrt, bias=eps_q[:])
                    # k: 1/sqrt(sum/DH + eps)
                    nc.scalar.activation(rms[:, 8:16], ms[:, 8:16], AF.Sqrt,
                                         bias=eps_k[:], scale=1.0 / DH)
                    alpha = p1s.tile([128, 16], f32, tag="alpha")
                    nc.vector.reciprocal(alpha[:], rms[:])
                    # fold the exp2 bit-trick scale into offloaded heads' q-alpha
                    assert OFF_HEADS == (1, 3, 5, 7)
                    off_ap = APX(alpha[:], [alpha[:].ap[0], [2, 4]], 1)
                    nc.vector.tensor_scalar_mul(off_ap, off_ap, A_SCALE)

                    # RoPE + alpha scaling.  qk cols: q = [0:576), k = [576:1152)
                    def rope(base_off, cos_t, sin_t, alpha_sl, out_sl, eng):
                        tmp = p1s.tile([128, CPC], f32, tag="ropetmp%d" % base_off)
                        rot = p1s.tile([128, CPC], f32, tag="roterot%d" % base_off)
                        qk0 = qk[:, base_off : base_off + CPC]
                        p_tmp, p_qk = tmp[:].ap[0], qk0.ap[0]
                        p_cos, p_sin = cos_t.ap[0], sin_t.ap[0]
                        p_al, p_out = alpha_sl.ap[0], out_sl.ap[0]
                        # tmp[h,0:36] = x2 * (-sin) ; tmp[h,36:72] = x1 * (+sin)
                        eng.tensor_tensor(
                            APX(tmp[:], [p_tmp, [DH, HPC], [1, HALF]]),
                            APX(qk0, [p_qk, [DH, HPC], [1, HALF]], HALF),
                            APX(sin_t, [p_sin, [0, HPC], [1, HALF]]),
                            op=ALU.mult,
                        )
                        eng.tensor_tensor(
                            APX(tmp[:], [p_tmp, [DH, HPC], [1, HALF]], HALF),
                            APX(qk0, [p_qk, [DH, HPC], [1, HALF]]),
                            APX(sin_t, [p_sin, [0, HPC], [1, HALF]], HALF),
                            op=ALU.mult,
                        )
                        eng.tensor_tensor(
                            rot[:].rearrange("p (h d) -> p h d", h=HPC),
                            qk0.rearrange("p (h d) -> p h d", h=HPC),
                            APX(cos_t, [p_cos, [0, HPC], [1, DH]]),
                            op=ALU.mult,
                        )
                        eng.tensor_tensor(rot[:], rot[:], tmp[:], op=ALU.add)
                        eng.tensor_tensor(
                            out_sl.rearrange("p (h d) -> p h d", h=HPC),
                            rot[:].rearrange("p (h d) -> p h d", h=HPC),
                            APX(alpha_sl, [p_al, [1, HPC], [0, DH]]),
                            op=ALU.mult,
                        )

                    qhat_t = p1s.tile([128, CPC], bf16, tag="qhat")
                    rope(0, cosq[:, it, :], sinq[:, it, :], alpha[:, 0:8],
                         qhat_t[:], nc.vector)
                    khat = p1s.tile([128, CPC], bf16, tag="khat")
                    rope(CPC, cosk[:, it, :], sink[:, it, :], alpha[:, 8:16],
                         khat[:], nc.gpsimd)
                    if len(pend) > 1:
                        emit_ktr(*pend.pop(0))

                    # PE-transpose k-hat per head -> khT bf16 (deferred one
                    # tile so PE never waits on this tile's rope chain)
                    def emit_ktr(it_, khat_, qhat_ref_):
                        for dst, nat, cptag in ((khT, khat_, 0), (qT, qhat_ref_, 1)):
                            for hb in (0, 4):
                                tp = trps.tile([DH, 4, 128], bf16,
                                               tag="ktr%d" % cptag,
                                               name="ktr%d_%d_%d" % (cptag, it_, hb))
                                for h4 in range(4):
                                    src_ap = nat[:, (hb + h4) * DH : (hb + h4 + 1) * DH]
                                    nc.tensor.transpose(tp[:, h4, :], src_ap, ident[:])
                                if cptag == 0:
                                    nc.scalar.copy(
                                        dst[0:DH, hb : hb + 4, it_ * 128 : (it_ + 1) * 128],
                                        tp[:],
                                    )
                                else:
                                    nc.vector.tensor_copy(
                                        dst[0:DH, hb : hb + 4, it_ * 128 : (it_ + 1) * 128],
                                        tp[:],
                                    )
                    pend.append((it, khat, qhat_t))

                for _args in pend:
                    emit_ktr(*_args)

            for _cb in range(PCB):
                nc.gpsimd.dma_start(wproj[:, _cb], wproj_d[:, _cb])

            # ------------------------------------------------ phase 2
            with tc.tile_pool(name="p2", bufs=2) as p2, \
                 tc.tile_pool(name="p2o", bufs=4) as p2o, \
                 tc.tile_pool(name="pint", bufs=3) as pints, \
                 tc.tile_pool(name="sps", bufs=2, space="PSUM") as sps, \
                 tc.tile_pool(name="pvps", bufs=2, space="PSUM") as pvps, \
                 tc.tile_pool(name="yps", bufs=2, space="PSUM") as yps, \
                 tc.tile_pool(name="dram", bufs=1, space="DRAM") as dpool:
                rec_dram = dpool.tile([NJ, HPC, TQ], f32)
                rec2_dram = dpool.tile([NJ, HPC, TQ], f32)

                def emit_proj_group(proj_in_, j_, ts, e):
                    yp = yps.tile([128, ECH], f32, tag="yp",
                                  name="yp%d_%d_%d" % (j_, ts, e))
                    for cb in range(PCB):
                        rows = 128 if cb < PCB - 1 else CPC - 128 * (PCB - 1)
                        nc.tensor.matmul(
                            yp[:],
                            lhsT=proj_in_[0:rows, cb, ts * 128 : (ts + 1) * 128],
                            rhs=wproj[0:rows, cb, e * ECH : (e + 1) * ECH],
                            start=(cb == 0), stop=(cb == PCB - 1),
                        )
                    ysb = p2o.tile([128, ECH], f32, tag="ysb",
                                   name="ysb%d_%d_%d" % (j_, ts, e))
                    nc.vector.tensor_copy(ysb[:], yp[:])
                    for _yh in range(2):
                        nc.sync.dma_start(
                            y_d[j_ * TQ + ts * 128 : j_ * TQ + (ts + 1) * 128,
                                e * ECH + _yh * (ECH // 2)
                                : e * ECH + (_yh + 1) * (ECH // 2)],
                            ysb[:, _yh * (ECH // 2) : (_yh + 1) * (ECH // 2)],
                        )

                proj_pend = []
                proj_ins = {}
                pv_tiles = {}

                def finish_head(j, h):
                    # normalize: row 96 of pv is the softmax denominator
                    pv = pv_tiles.pop((j, h))
                    proj_in = proj_ins[j]
                    pvs = p2o.tile([73, TQ], f32, tag="pvs")
                    nc.vector.tensor_copy(pvs[:], pv[:])
                    nrm = p2o.tile([73, TQ], f32, tag="nrm")
                    # reshape the denominator row to [8,64] via DRAM so the
                    # microcoded reciprocal runs 8-wide (~6x cheaper on DVE)
                    nc.sync.dma_start(rec_dram[j, h, :], pvs[72:73, :])
                    r8 = p2o.tile([8, 64], f32, tag="r8")
                    r8o = p2o.tile([8, 64], f32, tag="r8o")
                    nc.sync.dma_start(
                        r8[:], APX(rec_dram[j, h, :], [[64, 8], [1, 64]]))
                    nc.vector.reciprocal(r8o[:], r8[:])
                    nc.sync.dma_start(
                        APX(rec2_dram[j, h, :], [[64, 8], [1, 64]]), r8o[:])
                    bc = nrm[0:DH, :]
                    nc.sync.dma_start(
                        bc,
                        APX(rec2_dram[j, h, :], [[0, DH], [1, TQ]]),
                    )
                    outT = p2o.tile([DH, TQ], bf16, tag="outT")
                    nc.gpsimd.tensor_tensor(outT[:], pvs[0:DH, :], bc,
                                            op=ALU.mult)
                    # repack head rows into 128-row proj blocks (SBUF->SBUF DMA)
                    r0 = h * DH
                    cb0, off0 = divmod(r0, 128)
                    n0 = min(DH, 128 - off0)
                    nc.gpsimd.dma_start(
                        proj_in[off0 : off0 + n0, cb0, :], outT[0:n0, :]
                    )
                    if n0 < DH:
                        nc.gpsimd.dma_start(
                            proj_in[0 : DH - n0, cb0 + 1, :], outT[n0:DH, :]
                        )
                    if h == HPC - 1:
                        # queue this q-chunk's projection; drained next chunk
                        for ts in range(4):
                            for e in range(C // ECH):
                                proj_pend.append((proj_in, j, ts, e))

                def emit_pv(j, h, gg, pbuf):
                    pv = pv_tiles[(j, h)]
                    for ii in range(2):
                        i = gg * 2 + ii
                        nc.tensor.matmul(
                            pv[:],
                            lhsT=vaug[:, i, h, :],
                            rhs=pbuf[:, ii, :],
                            start=(i == 0), stop=(i == 15),
                            skip_group_check=True,
                        )
                    if gg == 7:
                        finish_head(j, h)

                # pair-interleaved software-pipelined stream: heads (2hp,
                # 2hp+1) alternate per k-pair so the even head's exp
                # (ScalarE) and the odd head's exp2 bit-trick (DVE int32
                # convert + GpSimd bitcast copy) run on disjoint engines.
                # PV for step n is deferred 3 steps to cover the act-chain
                # latency, keeping the PE streaming back-to-back.
                stream = []
                for j in range(NJ):
                    for hp in range(HPC // 2):
                        for gg in range(8):
                            stream.append((j, 2 * hp, gg))
                            stream.append((j, 2 * hp + 1, gg))
                pend_pv = []
                for (j, h, gg) in stream:
                    if h == 0 and gg == 0:
                        proj_ins[j] = p2.tile([128, PCB, TQ], bf16,
                                              tag="proj_in",
                                              name="proj_in%d" % j)
                    if gg == 0:
                        pv_tiles[(j, h)] = pvps.tile(
                            [73, TQ], f32, tag="pv", name="pv%d_%d" % (j, h))
                    sp = sps.tile([128, 2, TQ], f32, tag="sp")
                    koff = KAUG if h in OFF_HEADS else DH
                    for ii in range(2):
                        i = gg * 2 + ii
                        nc.tensor.matmul(
                            sp[:, ii, :],
                            lhsT=khT[0:koff, h, i * 128 : (i + 1) * 128],
                            rhs=qT[0:koff, h, j * TQ : (j + 1) * TQ],
                            start=True, stop=True,
                        )
                    if h in OFF_HEADS:
                        pi = pints.tile([128, 2, TQ], i16, tag="pi")
                        nc.vector.tensor_copy(pi[:], sp[:])
                        pbuf = pi.bitcast(bf16)
                    else:
                        pbuf = p2o.tile([128, 2, TQ], bf16, tag="pbuf")
                        nc.scalar.activation(
                            pbuf[:].rearrange("p a b -> p (a b)"),
                            sp[:].rearrange("p a b -> p (a b)"),
                            AF.Exp,
                        )
                    pend_pv.append((j, h, gg, pbuf))
                    if len(pend_pv) > 3:
                        emit_pv(*pend_pv.pop(0))
                    # spread deferred proj groups into the stream
                    if h % 2 == 1 and gg in (2, 4, 6) and proj_pend:
                        emit_proj_group(*proj_pend.pop(0))
                for _a in pend_pv:
                    emit_pv(*_a)
                # dummy matmuls bridge the ~8us norm-chain latency before the
                # last chunk's proj groups, keeping the PE at full p-state
                for _w in range(16):
                    warm = sps.tile([128, 2, TQ], f32, tag="sp",
                                    name="warm%d" % _w)
                    nc.tensor.matmul(
                        warm[:, 0, :],
                        lhsT=khT[0:DH, 0, 0:128],
                        rhs=qT[0:DH, 0, 0:TQ],
                        start=True, stop=True,
                    )
                for args in proj_pend:
                    emit_proj_group(*args)

    nc.compile()
    _NC = nc
    return nc


# -------------------------------------------------------------- host prep
def _prep_shards(x, w_qkv, w_proj, q_norm_w, k_norm_w):
    inv_freq = 1.0 / (THETA ** (np.arange(HALF, dtype=np.float32) / HALF))
    ang = np.arange(N, dtype=np.float32)[:, None] * inv_freq[None, :]
    cos_t, sin_t = np.cos(ang), np.sin(ang)  # [N, 36]

    def rope_tabs(w):
        # cos2[t, j] = cos(ang) * w[j] (both halves); sin2s = [-sin, +sin] * w
        c2 = np.concatenate([cos_t * w[:HALF], cos_t * w[HALF:]], axis=1)
        s2 = np.concatenate([-sin_t * w[:HALF], sin_t * w[HALF:]], axis=1)
        tile_form = lambda a: np.ascontiguousarray(
            a.reshape(NT, 128, DH).transpose(1, 0, 2)
        ).astype(np.float32)
        return tile_form(c2), tile_form(s2)

    cq, sq_ = rope_tabs(np.asarray(q_norm_w, np.float32))
    ck, sk = rope_tabs(np.asarray(k_norm_w, np.float32))

    augk = np.ones((2, HPC, N), dtype=_BF16)
    augq = np.empty((2, HPC, N), dtype=_BF16)
    augq[0] = _BF16(B_HI)
    augq[1] = _BF16(B_LO)

    xTs = []
    for b in range(B):
        xt = np.ascontiguousarray(x[b].T)  # [1152, 2048]
        xt = xt.reshape(NCCH, 128, NT, 128).transpose(1, 2, 0, 3)
        xTs.append(np.ascontiguousarray(xt).astype(_BF16))

    in_maps = []
    for core in range(8):
        b, g = divmod(core, 2)
        h0 = g * HPC
        rq = w_qkv[h0 * DH : h0 * DH + CPC]                     # [576, 1152]
        rk = w_qkv[C + h0 * DH : C + h0 * DH + CPC]
        rv = w_qkv[2 * C + h0 * DH : 2 * C + h0 * DH + CPC]
        wk = np.concatenate([rq, rk, rv], axis=0).T             # [1152, 1728]
        wk = wk.reshape(NCCH, 128, 3 * CPC).transpose(1, 0, 2)
        wk = np.ascontiguousarray(wk).astype(_BF16)

        wp = w_proj[:, g * CPC : (g + 1) * CPC].T               # [576, 1152]
        wp = np.concatenate(
            [wp, np.zeros((PCB * 128 - CPC, C), np.float32)], axis=0
        )
        wp = wp.reshape(PCB, 128, C).transpose(1, 0, 2)
        wp = np.ascontiguousarray(wp).astype(_BF16)

        in_maps.append({
            "xT": xTs[b], "wqkv": wk, "wproj": wp,
            "cosq": cq, "sinq": sq_, "cosk": ck, "sink": sk,
            "augk": augk, "augq": augq,
        })
    return in_maps


def kernel(x, w_qkv, w_proj, b_proj, q_norm_w, k_norm_w):
    x = np.asarray(x, np.float32)
    w_qkv = np.asarray(w_qkv, np.float32)
    w_proj = np.asarray(w_proj, np.float32)
    b_proj = np.asarray(b_proj, np.float32)

    nc = _build()
    from concourse.bass_utils import run_bass_kernel_spmd

    in_maps = _prep_shards(x, w_qkv, w_proj, q_norm_w, k_norm_w)
    res = run_bass_kernel_spmd(nc, in_maps, core_ids=list(range(8)))
    y = np.empty((B, N, C), np.float32)
    for b in range(B):
        y[b] = res.results[2 * b]["y"] + res.results[2 * b + 1]["y"] + b_proj
    return y

